# revision 26
# baseline (speedup 1.0000x reference)
"""GQA attention block (Wq/Wk/Wv -> RoPE -> softmax(QK^T)V -> Wo) on 8 Trainium2
NeuronCores.

Sharding (tensor-parallel per the head-sharding scheme):
  core c in 0..7: batch b = c // 4, head-group g = c % 4.
  Each core owns 8 q-heads (global 8g..8g+7) and 2 kv-heads (2g, 2g+1) of one
  batch element, computes its slice of q/k/v projections, RoPE, attention, and
  a partial o_proj (Wo rows for its heads). The all-reduce after o_proj is the
  host-side unshard: out[b] = sum of the 4 partial outputs of batch b.

On-device layout (per core), everything feature-on-partitions ("transposed"):
  xt    [D=2048, S=2048]   x^T for this batch (persistent; q-proj reads it
                           during attention)
  QT    [E=512,  S]        q^T; partition-tile j holds head pair (j, j+4):
                           local head j (kv0) on partitions 0:64, head j+4
                           (kv1) on partitions 64:128. Wq columns are permuted
                           on the host to produce this layout directly.
  KT    [128, S]           k^T; kv0 on partitions 0:64, kv1 on 64:128.
  V     [S, 130] as 16 tiles [128, 130]: cols 0:64 v(kv0), col 64 ones,
                           cols 65:129 v(kv1), col 129 ones  (v_aug).
  scores^T per head: [sk, sq] so exp is ACT psum->sbuf and the attn@v
  contraction (over sk) uses v_aug as the stationary operand; row 64 of the
  attn@v output is the softmax denominator (ones column trick).

Schedule: the ACT engine (exp over all S*S scores) is the global bottleneck
(~256 exps x ~1.0us). Scope 1 computes only K/V projections (+RoPE/transpose)
for all chunks (~35us). Scope 2 runs a FLAT software pipeline over all 256
(pair, sq-chunk, sk-tile) steps: per step ACT does one exp while the PE does
av(step-2) + scores(step+2) (cross-pair pre-issue keeps ACT saturated at
boundaries) + budgeted fill-in work popped from a queue (q-projection of
later pairs, o_proj of finished chunks, softmax normalization).

RoPE is DVE-only: rotate_half is a fixed +-32-partition shift, so tsin is
computed in four partition-shifted tensor_mul slices against a sign-folded
sin table; no PE rotation matmul, no PSUM->SBUF raw copy on ACT.

Matmuls run in bf16 (PSUM accumulates fp32; ~5e-3 rel err vs the 2e-2 gate).
"""

import sys

if "/opt/trn_rl_repo" not in sys.path:
    sys.path.insert(0, "/opt/trn_rl_repo")

from collections import deque
from contextlib import ExitStack

import ml_dtypes
import numpy as np

import concourse.bass as bass  # noqa: F401  (engine types via nc)
import concourse.tile as tile
from concourse import bacc, bass_utils, mybir

F32 = mybir.dt.float32
F32R = mybir.dt.float32r
BF16 = mybir.dt.bfloat16
AF = mybir.ActivationFunctionType
NP_BF16 = ml_dtypes.bfloat16

# Problem constants (hardcoded per harness contract)
B = 2
S = 2048  # sequence length
D = 2048  # d_model
N_HEADS = 32
N_KV = 8
HD = 64  # head dim
ROPE_BASE = 500000.0
N_CORES = 8

# Per-core derived
NQ = N_HEADS // 4  # 8 local q heads (4 head-groups)
E = NQ * HD  # 512 local q features
NPAIR = NQ // 2  # 4 head pairs / e-tiles
KVW = 2 * HD  # 128 local kv features
SC = 512  # s-chunk (projection + sq chunk)
NSC = S // SC  # 4
DT = D // 128  # 16 d-tiles
SKT = S // 128  # 16 sk tiles
ET = E // 128  # 4 e-tiles
SCALE = 1.0 / float(np.sqrt(HD))
NSTEP = NPAIR * NSC * SKT  # 256 flat attention steps


def build_program():
    nc = bacc.Bacc(
        "TRN2", target_bir_lowering=False, debug=False, enable_asserts=False
    )

    xt = nc.dram_tensor("xt", [D, S], BF16, kind="ExternalInput").ap()
    wq = nc.dram_tensor("wq", [D, E], BF16, kind="ExternalInput").ap()
    wk = nc.dram_tensor("wk", [D, KVW], BF16, kind="ExternalInput").ap()
    wv = nc.dram_tensor("wv", [D, KVW], BF16, kind="ExternalInput").ap()
    wo = nc.dram_tensor("wo", [E, D], BF16, kind="ExternalInput").ap()
    cosd = nc.dram_tensor("cosd", [128, S], BF16, kind="ExternalInput").ap()
    sinnd = nc.dram_tensor("sinnd", [128, S], BF16, kind="ExternalInput").ap()
    ident = nc.dram_tensor("ident", [128, 128], BF16, kind="ExternalInput").ap()
    ones1 = nc.dram_tensor("ones1", [1, 128], F32R, kind="ExternalInput").ap()
    onesc = nc.dram_tensor("onesc", [128, 1], BF16, kind="ExternalInput").ap()
    out = nc.dram_tensor("out", [S, D], F32, kind="ExternalOutput").ap()

    wq_r = wq.rearrange("(t p) e -> p t e", p=128)
    wk_r = wk.rearrange("(t p) e -> p t e", p=128)
    wv_r = wv.rearrange("(t p) e -> p t e", p=128)
    wo_r = wo.rearrange("(t p) d -> p t d", p=128)
    xt_r = xt.rearrange("(t p) s -> p t s", p=128)

    with tile.TileContext(nc) as tc, ExitStack() as ctx:
        persist = ctx.enter_context(tc.tile_pool(name="persist", bufs=1))

        # Persistent SBUF state
        qt_sb = [persist.tile([128, S], BF16, tag=f"qt{j}", name=f"qt{j}") for j in range(NPAIR)]
        kt_sb = persist.tile([128, S], BF16, tag="kt")
        v_sb = [persist.tile([128, 130], BF16, tag=f"v{j}", name=f"v{j}") for j in range(SKT)]
        xt_sb = persist.tile([128, DT, S], BF16, tag="xts")
        wq_sb = persist.tile([128, DT, E], BF16, tag="wq")
        wk_sb = persist.tile([128, DT, KVW], BF16, tag="wk")
        wv_sb = persist.tile([128, DT, KVW], BF16, tag="wv")
        cos_sb = persist.tile([128, S], BF16, tag="cos")
        sinn_sb = persist.tile([128, S], BF16, tag="sinn")
        ident_sb = persist.tile([128, 128], BF16, tag="ident")
        ones1_sb = persist.tile([1, 128], F32R, tag="ones1")
        onesc_sb = persist.tile([128, 1], BF16, tag="onesc")
        nc.scalar.dma_start(out=ident_sb, in_=ident)
        nc.scalar.dma_start(out=ones1_sb, in_=ones1)
        nc.scalar.dma_start(out=onesc_sb, in_=onesc)

        # ---- input DMA: per-tile transfers, SP + Pool rings (ACT stays free
        # for compute). wk + xt chunk 0 + cos/sinneg chunk 0 gate the start.
        for g in range(4):
            nc.sync.dma_start(
                out=wk_sb[:, 4 * g : 4 * (g + 1), :],
                in_=wk_r[:, 4 * g : 4 * (g + 1), :],
            )
        nc.gpsimd.dma_start(out=cos_sb[:, 0:SC], in_=cosd[:, 0:SC])
        nc.gpsimd.dma_start(out=sinn_sb[:, 0:SC], in_=sinnd[:, 0:SC])
        for c in range(NSC):
            for t in range(DT):
                (nc.sync if t % 2 == 0 else nc.gpsimd).dma_start(
                    out=xt_sb[:, t, bass.ts(c, SC)],
                    in_=xt_r[:, t, bass.ts(c, SC)],
                )
            if c == 0:
                for g in range(1, 4):
                    nc.gpsimd.dma_start(
                        out=cos_sb[:, bass.ts(g, SC)], in_=cosd[:, bass.ts(g, SC)]
                    )
                    nc.gpsimd.dma_start(
                        out=sinn_sb[:, bass.ts(g, SC)], in_=sinnd[:, bass.ts(g, SC)]
                    )
                for g in range(4):
                    nc.gpsimd.dma_start(
                        out=wv_sb[:, 4 * g : 4 * (g + 1), :],
                        in_=wv_r[:, 4 * g : 4 * (g + 1), :],
                    )
            if c == 1:
                for t in range(DT):
                    (nc.sync if t % 2 else nc.gpsimd).dma_start(
                        out=wq_sb[:, t, :], in_=wq_r[:, t, :]
                    )

        def rope_dve(dst, src_ps, c, tmp_pool):
            """dst[:, c*SC:+SC] = src*cos + rot(src)*sin, DVE-only.
            rot is a +-32 partition shift; sinn_sb has the sign folded in
            (rows 0:32 and 64:96 hold -sin)."""
            sl = bass.ts(c, SC)
            tcos = tmp_pool.tile([128, SC], F32, tag="tmp", name="tcos", bufs=4)
            nc.vector.tensor_mul(tcos, src_ps, cos_sb[:, sl])
            tsin = tmp_pool.tile([128, SC], F32, tag="tmp", name="tsin", bufs=4)
            for blk, src_blk in ((0, 1), (1, 0), (2, 3), (3, 2)):
                nc.vector.tensor_mul(
                    tsin[bass.ts(blk, 32), :],
                    src_ps[bass.ts(src_blk, 32), :],
                    sinn_sb[bass.ts(blk, 32), sl],
                )
            nc.vector.tensor_add(dst[:, sl], tcos, tsin)

        # ---------------- Scope 1: K/V projections, RoPE, V transpose --------
        with (
            tc.tile_pool(name="p1st", bufs=2) as p1st,
            tc.tile_pool(name="kv_ps", bufs=2, space="PSUM") as kv_ps,
            tc.tile_pool(name="tr_ps", bufs=2, space="PSUM") as tr_ps,
        ):
            for c in range(NSC):
                kp = kv_ps.tile([128, SC], F32, tag="kv", name="kp")
                for t in range(DT):
                    nc.tensor.matmul(
                        kp,
                        wk_sb[:, t, :],
                        xt_sb[:, t, bass.ts(c, SC)],
                        start=(t == 0),
                        stop=(t == DT - 1),
                    )
                rope_dve(kt_sb, kp, c, p1st)

                vp = kv_ps.tile([128, SC], F32, tag="kv", name="vp")
                for t in range(DT):
                    nc.tensor.matmul(
                        vp,
                        wv_sb[:, t, :],
                        xt_sb[:, t, bass.ts(c, SC)],
                        start=(t == 0),
                        stop=(t == DT - 1),
                    )
                vt_sb = p1st.tile([128, SC], BF16, tag="vtsb", bufs=2)
                nc.vector.tensor_copy(vt_sb, vp)
                for ss in range(SC // 128):
                    sk = c * (SC // 128) + ss
                    tp = tr_ps.tile([128, 128], BF16, tag="tr")
                    nc.tensor.transpose(tp, vt_sb[:, bass.ts(ss, 128)], ident_sb)
                    nc.vector.tensor_copy(v_sb[sk][:, 0:64], tp[:, 0:64])
                    nc.vector.tensor_copy(v_sb[sk][:, 65:129], tp[:, 64:128])
                    nc.vector.tensor_copy(v_sb[sk][:, 64:65], onesc_sb)
                    nc.vector.tensor_copy(v_sb[sk][:, 129:130], onesc_sb)

        # ---------------- Scope 2: flat attention pipeline -------------------
        with (
            tc.tile_pool(name="wop", bufs=1) as wop,
            tc.tile_pool(name="attnp", bufs=1) as attnp,
            tc.tile_pool(name="expp", bufs=6) as expp,
            tc.tile_pool(name="recp", bufs=4) as recp,
            tc.tile_pool(name="ostg", bufs=3) as ostg,
            tc.tile_pool(name="p2st", bufs=2) as p2st,
            tc.tile_pool(name="sc_ps", bufs=2, space="PSUM") as sc_ps,
            tc.tile_pool(name="av_ps", bufs=1, space="PSUM") as av_ps,
            tc.tile_pool(name="qt_ps", bufs=1, space="PSUM") as qt_ps,
            tc.tile_pool(name="sh_ps", bufs=1, space="PSUM") as sh_ps,
        ):
            wo_sb = wop.tile([128, ET, D], BF16, tag="wo")
            for t in range(ET):
                for h in range(2):
                    nc.gpsimd.dma_start(
                        out=wo_sb[:, t, bass.ts(h, D // 2)],
                        in_=wo_r[:, t, bass.ts(h, D // 2)],
                    )
            attn_sb = [attnp.tile([128, S], BF16, tag=f"at{j}", name=f"at{j}") for j in range(NPAIR)]

            work = deque()  # (cost_ns, closure, key) fill-in work for PE slack
            qp_left = {}  # pair -> # of unemitted qproj items (emission barrier)

            def qproj(pc):
                """Q projection + rope for pair-chunk pc as queued work items."""
                cs, j = divmod(pc, NPAIR)
                state = {}

                def start_mm(t0, t1):
                    def run():
                        if "qp" not in state:
                            state["qp"] = qt_ps.tile(
                                [128, SC], F32, tag="qp", name="qp"
                            )
                        for t in range(t0, t1):
                            nc.tensor.matmul(
                                state["qp"],
                                wq_sb[:, t, bass.ts(j, 128)],
                                xt_sb[:, t, bass.ts(cs, SC)],
                                start=(t == 0),
                                stop=(t == DT - 1),
                            )

                    return run

                for t0 in range(0, DT, 2):
                    work.append((430, start_mm(t0, t0 + 2), pc))
                work.append(
                    (0, lambda: rope_dve(qt_sb[j], state["qp"], cs, p2st), pc)
                )
                qp_left[pc] = DT // 2 + 1

            def make_normalize(attn_slice, den, half):
                def run():
                    rec32 = recp.tile([1, SC], F32, tag="rec32", name="rec32")
                    nc.vector.reciprocal_approx_fast(rec32, den)
                    rec = recp.tile([1, SC], F32R, tag="rec")
                    nc.vector.tensor_copy(rec, rec32)
                    bp = sh_ps.tile([128, SC], F32, tag="sh", name="bp")
                    nc.tensor.matmul(bp, ones1_sb, rec, start=True, stop=True)
                    nc.vector.tensor_mul(
                        attn_slice, attn_slice, bp[bass.ds(64 * half, 64), :]
                    )

                return run

            def oproj(cs):
                """o_proj for sq chunk cs (all 4 sq-subtiles x 4 D-chunks)."""
                for st_local in range(4):
                    st = cs * 4 + st_local
                    ot = ostg.tile([128, SC], F32, tag="ostg", name="ostg")
                    state = {}

                    def start_mm(mc, t0, t1, st=st, state=state):
                        def run():
                            if "op" not in state:
                                state["op"] = sh_ps.tile(
                                    [128, SC], F32, tag="sh", name="op"
                                )
                            for t in range(t0, t1):
                                nc.tensor.matmul(
                                    state["op"],
                                    attn_sb[t][:, bass.ts(st, 128)],
                                    wo_sb[:, t, bass.ts(mc, SC)],
                                    start=(t == 0),
                                    stop=(t == ET - 1),
                                )

                        return run

                    def finish(mc, ot=ot, st=st, state=state):
                        def run():
                            op = state.pop("op")
                            nc.vector.tensor_copy(ot, op)
                            nc.sync.dma_start(
                                out=out[bass.ts(st, 128), bass.ts(mc, SC)],
                                in_=ot,
                            )

                        return run

                    for mc in range(D // SC):
                        work.append((430, start_mm(mc, 0, 2), None))
                        work.append((450, start_mm(mc, 2, 4), None))
                        work.append((0, finish(mc), None))

            # Flat pipeline state
            sc_tiles = {}
            exp_tiles = {}
            av_tiles = {}

            def scores(gs):
                p, jj = divmod(gs, SKT)
                cs, j = divmod(p, NPAIR)
                sq = bass.ts(cs, SC)
                t = sc_ps.tile([128, 2 * SC], F32, tag="sc", name="sc")
                sc_tiles[gs] = t
                nc.tensor.matmul(
                    t[:, 0:SC],
                    kt_sb[0:64, bass.ts(jj, 128)],
                    qt_sb[j][0:64, sq],
                    start=True,
                    stop=True,
                    tile_position=(0, 0),
                )
                nc.tensor.matmul(
                    t[:, SC : 2 * SC],
                    kt_sb[64:128, bass.ts(jj, 128)],
                    qt_sb[j][64:128, sq],
                    start=True,
                    stop=True,
                    tile_position=(64, 0),
                )

            def exp_step(gs):
                et = expp.tile([128, 2 * SC], BF16, tag="exp")
                exp_tiles[gs] = et
                nc.scalar.activation(et, sc_tiles.pop(gs), AF.Exp, scale=SCALE)

            def av_step(gs):
                p, jj = divmod(gs, SKT)
                if jj == 0:
                    av_tiles[p] = (
                        av_ps.tile([65, SC], F32, tag="ava", name="ava"),
                        av_ps.tile([65, SC], F32, tag="avb", name="avb"),
                    )
                av_a, av_b = av_tiles[p]
                et = exp_tiles.pop(gs)
                nc.tensor.matmul(
                    av_a,
                    v_sb[jj][:, 0:65],
                    et[:, 0:SC],
                    start=(jj == 0),
                    stop=(jj == SKT - 1),
                )
                nc.tensor.matmul(
                    av_b,
                    v_sb[jj][:, 65:130],
                    et[:, SC : 2 * SC],
                    start=(jj == 0),
                    stop=(jj == SKT - 1),
                )

            def pair_end(p):
                cs, j = divmod(p, NPAIR)
                sq = bass.ts(cs, SC)
                av_a, av_b = av_tiles.pop(p)
                for half, av in ((0, av_a), (1, av_b)):
                    attn_slice = attn_sb[j][bass.ds(64 * half, 64), sq]
                    nc.vector.tensor_copy(attn_slice, av[0:64, :])
                    den = recp.tile([1, SC], F32, tag="den", name="den")
                    nc.vector.tensor_copy(den, av[64:65, :])
                    work.append((430, make_normalize(attn_slice, den, half), None))
                if j == NPAIR - 1:
                    oproj(cs)

            def pop_one():
                cost, fn, key = work.popleft()
                fn()
                if key is not None:
                    qp_left[key] -= 1
                return cost

            def drain_for_pair(p):
                # Emission barrier: all qproj(p) instructions must be emitted
                # before pair p's first score reads qt_sb (the tile framework
                # orders dependencies by emission order).
                while qp_left.get(p, 0) > 0:
                    pop_one()

            # Prime: q-proj for pairs 0/1, then the first two score steps.
            qproj(0)
            qproj(1)
            while work:
                pop_one()
            scores(0)
            scores(1)
            qproj(2)

            BUDGET = 500  # ns of queued PE work per step
            for gs in range(NSTEP + 2):
                if gs < NSTEP:
                    exp_step(gs)
                if gs >= 2:
                    av_step(gs - 2)
                    if (gs - 2) % SKT == SKT - 1:
                        pair_end((gs - 2) // SKT)
                if gs % SKT == 0 and gs // SKT + 3 <= NSTEP // SKT - 1:
                    qproj(gs // SKT + 3)
                spent = 0
                while work and spent + work[0][0] <= BUDGET:
                    spent += pop_one()
                if gs + 2 < NSTEP:
                    drain_for_pair((gs + 2) // SKT)
                    scores(gs + 2)
            while work:
                pop_one()

    nc.compile()
    return nc


_PROGRAM = None


def _get_program():
    global _PROGRAM
    if _PROGRAM is None:
        _PROGRAM = build_program()
    return _PROGRAM


def _rope_tables():
    inv_freq = 1.0 / (ROPE_BASE ** (np.arange(0, HD, 2, dtype=np.float32) / HD))
    t = np.arange(S, dtype=np.float32)
    freqs = np.outer(t, inv_freq)  # [S, 32]
    emb = np.concatenate([freqs, freqs], axis=-1)  # [S, 64]
    return np.cos(emb).astype(np.float32), np.sin(emb).astype(np.float32)


def _host_constants():
    cos_t, sin_t = _rope_tables()  # [S, 64]
    idx = np.arange(128) % HD
    cosd = np.ascontiguousarray(cos_t[:, idx].T).astype(NP_BF16)  # [128, S]
    sinn = np.ascontiguousarray(sin_t[:, idx].T)
    # Fold rotate_half's signs into the sin table: rows 0:32 and 64:96
    # multiply partition-shifted values by -sin.
    sinn[0:32] *= -1.0
    sinn[64:96] *= -1.0
    sinnd = sinn.astype(NP_BF16)
    ident = np.eye(128, dtype=NP_BF16)
    ones1 = np.ones((1, 128), np.float32)
    onesc = np.ones((128, 1), NP_BF16)
    return cosd, sinnd, ident, ones1, onesc


def _core_inputs(x, Wq, Wk, Wv, Wo, consts, xt_by_batch, core):
    b, g = divmod(core, 4)
    cosd, sinnd, ident, ones1, onesc = consts

    wq_c = np.empty((D, E), NP_BF16)
    wo_c = np.empty((E, D), NP_BF16)
    for j in range(NPAIR):
        ha = 8 * g + j  # global head, kv-head 2g
        hb = 8 * g + j + 4  # global head, kv-head 2g+1
        wq_c[:, j * 128 : j * 128 + 64] = Wq[:, ha * HD : (ha + 1) * HD]
        wq_c[:, j * 128 + 64 : (j + 1) * 128] = Wq[:, hb * HD : (hb + 1) * HD]
        wo_c[j * 128 : j * 128 + 64, :] = Wo[ha * HD : (ha + 1) * HD, :]
        wo_c[j * 128 + 64 : (j + 1) * 128, :] = Wo[hb * HD : (hb + 1) * HD, :]
    wk_c = Wk[:, 2 * g * HD : 2 * g * HD + KVW].astype(NP_BF16)
    wv_c = Wv[:, 2 * g * HD : 2 * g * HD + KVW].astype(NP_BF16)

    return {
        "xt": xt_by_batch[b],
        "wq": wq_c,
        "wk": wk_c,
        "wv": wv_c,
        "wo": wo_c,
        "cosd": cosd,
        "sinnd": sinnd,
        "ident": ident,
        "ones1": ones1,
        "onesc": onesc,
    }


def make_in_maps(x, Wq, Wk, Wv, Wo):
    consts = _host_constants()
    xt_by_batch = [np.ascontiguousarray(x[b].T).astype(NP_BF16) for b in range(B)]
    return [
        _core_inputs(x, Wq, Wk, Wv, Wo, consts, xt_by_batch, c)
        for c in range(N_CORES)
    ]


def kernel(x, Wq, Wk, Wv, Wo, _trace=False, _trace_kwargs=None):
    x = np.asarray(x, np.float32)
    Wq = np.asarray(Wq, np.float32)
    Wk = np.asarray(Wk, np.float32)
    Wv = np.asarray(Wv, np.float32)
    Wo = np.asarray(Wo, np.float32)

    nc = _get_program()
    in_maps = make_in_maps(x, Wq, Wk, Wv, Wo)
    res = bass_utils.run_bass_kernel_spmd(
        nc,
        in_maps,
        core_ids=list(range(N_CORES)),
        trace=_trace,
        **(_trace_kwargs or {}),
    )
    outs = [r["out"] for r in res.results]
    full = np.empty((B, S, D), np.float32)
    for b in range(B):
        full[b] = outs[4 * b] + outs[4 * b + 1] + outs[4 * b + 2] + outs[4 * b + 3]
    if _trace:
        return full, res
    return full
